# revision 1
# baseline (speedup 1.0000x reference)
"""Trainium2 Bass kernel for nn_CrossAttention_DenseAVInteractions.

Math: the reference builds a cartesian KV grid kv[b,i,j] = pv[b,i] + pa[b,j]
over (N_v, N_a) and attends 64 queries against all N_v*N_a = 65536 keys.
Because the logits decompose as s[q,(i,j)] = (q.k_v[i]) + (q.k_a[j]), the
softmax over the product grid factorizes exactly:

    p[q,(i,j)] = softmax_i(q.k_v)[q,i] * softmax_j(q.k_a)[q,j]
    out[q]     = softmax_i(q.k_v) @ v_v + softmax_j(q.k_a) @ v_a

so the whole attention reduces to two 256-key attentions per (b, h).

Sharding (8 cores): core c handles batch b = c // 4 and the head pair
(2j, 2j+1) with j = c % 4.  Each core computes its heads' partial output
projection partial = out_heads @ Wproj[:, head_cols].T in f32; the host sums
the 4 partials per batch and adds bproj.

Device-side layout choices (from the first profile round):
 - All per-core inputs are packed on the host into ONE [128, 5376] f32
   tensor (contraction dim on partitions everywhere), loaded with 6 large
   DMAs split between the two HWDGE engines (sync + scalar) — per-DMA
   trigger cost is ~0.6us regardless of size, so few big transfers win.
 - All matmuls run with operands bitcast to float32r: same f32 bits, but
   1 cycle/row at free-dim >= 256 instead of fp32's two half-rate passes.
 - Softmax skips max-subtraction (logits here are ~N(0, 0.2^2), exp is
   safe) and uses the scalar engine's Exp activation with accum_out to
   get the denominator for free.
"""

import os
import sys

import numpy as np

sys.path.insert(0, "/opt/trn_rl_repo")

DIM = 512
H = 8
HD = DIM // H          # 64
B = 2
N_MM = 64
N_A = 256
N_V = 256
SCALE = HD ** -0.5     # 0.125
N_CORES = 8

# packA column offsets (f32 columns in the [128, 5376] packed input).
# Each HWDGE queue streams its half in consumption order; the v-weights and
# wproj land last because their dependent work (vT -> transpose -> PV, and
# the final projection) is the shortest tail.
O_WKV = 0         # 4 k-tiles x 128
O_XV = 512        # 4 x 256
O_WVV = 1536      # 4 x 128
O_WPROJ = 2048    # [128ch, 512]
O_WQ = 2560       # 4 x 128
O_XMM = 3072      # 4 x 64
O_WKA = 3328      # 4 x 128
O_XA = 3840       # 4 x 256
O_WVA = 4864      # 4 x 128
PACK_COLS = 5376

# chunk boundaries (cols) and which engine issues the load; per-engine
# emission order = HW queue FIFO order.  gpsimd (SWDGE) carries wvv so the
# two HWDGE queues finish earlier; xa is split across both HWDGE queues.
CHUNKS = [
    (0, 512, "sync"),        # wkv
    (2560, 3328, "scalar"),  # wq + xmm
    (512, 1024, "sync"),     # xv k0,k1
    (3328, 3840, "scalar"),  # wka
    (1024, 1536, "sync"),    # xv k2,k3
    (3840, 4352, "scalar"),  # xa k0,k1
    (1536, 2048, "sync"),    # wvv
    (4352, 4864, "scalar"),  # xa k2,k3
    (2048, 2560, "sync"),    # wproj
    (4864, 5376, "scalar"),  # wva
]

_cached = {}


def _build_program():
    import concourse.bacc as bacc
    from concourse import mybir
    from concourse.tile import TileContext

    f32 = mybir.dt.float32
    f32r = mybir.dt.float32r
    nc = bacc.Bacc(name="cross_attn_dense_av")

    packA = nc.dram_tensor("packA", [128, PACK_COLS], f32, kind="ExternalInput")
    out_d = nc.dram_tensor("out", [64, 512], f32, kind="ExternalOutput")
    ident_d = nc.inline_tensor(np.eye(128, dtype=np.float32), name="ident128")
    ident2_d = nc.inline_tensor(
        np.tile(np.eye(64, dtype=np.float32), (2, 1)), name="ident64x2"
    )

    from contextlib import ExitStack

    def r(ap):
        return ap  # fp32 everywhere: f32r measured at ~1.5e-4 rel err (too lossy)

    with TileContext(nc) as tc, ExitStack() as ctx:
        io = ctx.enter_context(tc.tile_pool(name="io", bufs=1))
        work = ctx.enter_context(tc.tile_pool(name="work", bufs=1))
        ps_mm = ctx.enter_context(tc.tile_pool(name="ps_mm", bufs=4, space="PSUM"))
        ps_spt = ctx.enter_context(tc.tile_pool(name="ps_spt", bufs=2, space="PSUM"))
        ps_o = ctx.enter_context(tc.tile_pool(name="ps_o", bufs=1, space="PSUM"))
        ps_f = ctx.enter_context(tc.tile_pool(name="ps_f", bufs=1, space="PSUM"))

        # ---- loads: few large DMAs, two HWDGE engines in parallel ----
        ident = io.tile([128, 128], f32, tag="ident")
        nc.gpsimd.dma_start(out=ident, in_=ident_d[:, :])
        ident2 = io.tile([128, 64], f32, tag="ident2")
        nc.gpsimd.dma_start(out=ident2, in_=ident2_d[:, :])
        chunk_t = {}
        for lo, hi, eng in CHUNKS:
            t = io.tile([128, hi - lo], f32, tag=f"c{lo}")
            getattr(nc, eng).dma_start(out=t, in_=packA[:, lo:hi])
            chunk_t[lo] = t

        # ---- PE warmup: ~8 bf16 matmuls on memset scratch trip the HAM
        #      clock gate to 8/8 while the input DMAs are still in flight ----
        bf16 = mybir.dt.bfloat16
        warm_sb = io.tile([128, 512], bf16, tag="warm_sb")
        nc.vector.memset(warm_sb, 0.5)
        warm_ps = ps_f.tile([128, 448], f32, tag="f_ps")
        for w in range(12):
            nc.tensor.matmul(
                warm_ps, warm_sb[:, 0:128], warm_sb[:, 0:448],
                start=(w == 0), stop=(w == 11),
            )

        def col(off, width):
            """AP slice of the packed input at absolute column offset."""
            for lo, hi, _ in CHUNKS:
                if lo <= off and off + width <= hi:
                    return chunk_t[lo][:, off - lo:off - lo + width]
            raise ValueError(f"span {off}:{off + width} crosses chunk boundary")

        # ---- pipelined compute, emitted in expected execution order ----
        def kproj(o_wk, o_x, side):
            """kT [128ch, 256tok] = Wk_side @ x_side.T"""
            kp = ps_mm.tile([128, 256], f32, tag="mm")
            for k in range(4):
                nc.tensor.matmul(
                    kp, col(o_wk + 128 * k, 128), col(o_x + 256 * k, 256),
                    start=(k == 0), stop=(k == 3),
                )
            ks = work.tile([128, 256], f32, tag=f"k_sb{side}")
            nc.vector.tensor_copy(ks, kp)
            return ks

        def vproj(o_wv, o_x, side):
            """v [128tok x 2tiles, 128ch] via vT matmul + PE transpose"""
            vp = ps_mm.tile([128, 256], f32, tag="mm")
            for k in range(4):
                nc.tensor.matmul(
                    vp, col(o_wv + 128 * k, 128), col(o_x + 256 * k, 256),
                    start=(k == 0), stop=(k == 3),
                )
            vTs = work.tile([128, 256], f32, tag=f"vT_sb{side}")
            nc.scalar.copy(vTs, vp)
            vt_ps = ps_mm.tile([128, 256], f32, tag="mm")
            for t in range(2):
                nc.tensor.transpose(
                    vt_ps[:, 128 * t:128 * t + 128],
                    vTs[:, 128 * t:128 * t + 128],
                    ident,
                )
            vs = work.tile([128, 2, 128], f32, tag=f"v_sb{side}")
            nc.vector.tensor_copy(vs, vt_ps.rearrange("p (t c) -> p t c", t=2))
            return vs

        def scores_softmax(ks, side):
            """scores (partitions = 64*h + q) + one full-width exp/normalize;
            returns normalized p [128, 256] for this side."""
            sp = ps_spt.tile([128, 256], f32, tag="spt")
            for h in range(2):
                hs = slice(64 * h, 64 * h + 64)
                nc.tensor.matmul(
                    sp[hs, :], q2T[hs, :], ks[hs, :],
                    start=True, stop=True, tile_position=(64 * h, 64 * h),
                )
            # softmax over keys (no max-subtraction: |s| < ~2 by construction)
            p = work.tile([128, 256], f32, tag=f"p{side}")
            zsum = work.tile([128, 1], f32, tag=f"zsum{side}")
            zrec = work.tile([128, 1], f32, tag=f"zrec{side}")
            nc.scalar.activation(
                p, sp, mybir.ActivationFunctionType.Exp, accum_out=zsum
            )
            nc.vector.reciprocal(zrec, zsum)
            nc.vector.tensor_scalar_mul(p, p, zrec)
            return p

        def ptrans(p, side):
            """transpose p [128(h,q), 256keys] -> per key-block [128keys, (h,q)]
            in one full-width PE transpose per 128-key block."""
            pt_ps = ps_spt.tile([128, 2, 128], f32, tag="spt")
            for t in range(2):
                nc.tensor.transpose(
                    pt_ps[:, t, :], p[:, 128 * t:128 * t + 128], ident
                )
            pt = work.tile([128, 2, 128], f32, tag=f"pt_sb{side}")
            (nc.vector.tensor_copy if side == 0 else nc.scalar.copy)(pt, pt_ps)
            return pt

        # v-side chain first (its data streams in first), a-side behind it
        k_v = kproj(O_WKV, O_XV, 0)

        q_ps = ps_mm.tile([128, 64], f32, tag="mm")
        for k in range(4):
            nc.tensor.matmul(
                q_ps, col(O_WQ + 128 * k, 128), col(O_XMM + 64 * k, 64),
                start=(k == 0), stop=(k == 3),
            )
        q2T = work.tile([128, 64], f32, tag="q2T")
        nc.scalar.mul(q2T, q_ps, SCALE)

        p_v = scores_softmax(k_v, 0)
        v_v = vproj(O_WVV, O_XV, 0)
        pt_v = ptrans(p_v, 0)

        k_a = kproj(O_WKA, O_XA, 1)
        p_a = scores_softmax(k_a, 1)
        pt_a = ptrans(p_a, 1)
        v_a = vproj(O_WVA, O_XA, 1)

        v_sb = [v_v, v_a]
        pt_sides = [pt_v, pt_a]

        # PV: o[128ch(2 heads), 64q] accumulated per head (col-tiled for h=1)
        o_ps = ps_o.tile([128, 64], f32, tag="o")
        for h in range(2):
            hs = slice(64 * h, 64 * h + 64)
            n = 0
            for side in range(2):
                for t in range(2):
                    nc.tensor.matmul(
                        o_ps[hs, :],
                        v_sb[side][:, t, hs],
                        pt_sides[side][:, t, 64 * h:64 * h + 64],
                        start=(n == 0), stop=(n == 3),
                        tile_position=(0, 64 * h),
                    )
                    n += 1
        o_sb = work.tile([128, 64], f32, tag="o_sb")
        nc.scalar.copy(o_sb, o_ps)

        # output projection partial: [64q, 512]
        f_ps = ps_f.tile([64, 512], f32, tag="f_ps")
        nc.tensor.matmul(f_ps, o_sb, col(O_WPROJ, 512), start=True, stop=True)
        f_sb = work.tile([64, 512], f32, tag="f_sb")
        nc.vector.tensor_copy(f_sb[:, 0:256], f_ps[:, 0:256])
        nc.scalar.copy(f_sb[:, 256:512], f_ps[:, 256:512])
        nc.sync.dma_start(out=out_d[:, 0:256], in_=f_sb[:, 0:256])
        nc.scalar.dma_start(out=out_d[:, 256:512], in_=f_sb[:, 256:512])

    nc.finalize()
    return nc


def _km(a):
    """[512, C] K-major -> [128, 4*C] (4 k-tiles side by side)."""
    c = a.shape[1]
    return a.reshape(4, 128, c).transpose(1, 0, 2).reshape(128, 4 * c)


def _shard_inputs(xmm, xa, xv, Wq, Wkv, Wproj):
    """Build the 8 per-core input maps (one packed [128, 5376] tensor each)."""
    in_maps = []
    for core in range(N_CORES):
        b, j = divmod(core, 4)
        r = slice(128 * j, 128 * j + 128)               # head-pair rows in [0,512)
        rv = slice(512 + 128 * j, 512 + 128 * j + 128)  # v rows in Wkv
        pack = np.concatenate(
            [
                _km(Wkv[r, :512].T),        # O_WKV
                _km(xv[b].T),               # O_XV
                _km(Wkv[rv, :512].T),       # O_WVV
                Wproj[:, 128 * j:128 * j + 128].T,  # O_WPROJ
                _km(Wq[r, :].T),            # O_WQ
                _km(xmm[b].T),              # O_XMM
                _km(Wkv[r, 512:].T),        # O_WKA
                _km(xa[b].T),               # O_XA
                _km(Wkv[rv, 512:].T),       # O_WVA
            ],
            axis=1,
        )
        assert pack.shape == (128, PACK_COLS)
        in_maps.append({"packA": np.ascontiguousarray(pack, np.float32)})
    return in_maps


def _get_program():
    if "nc" not in _cached:
        _cached["nc"] = _build_program()
    return _cached["nc"]


def _register_ntff_hook():
    """Best-effort: register the axon NTFF profile hook that the container's
    antenv stub doesn't provide, so run_bass_kernel_spmd(trace=True) can
    measure HW exec time. No-op on failure."""
    try:
        import types

        try:
            from antenv.axon_hooks import get_axon_ntff_profile_hook
            if get_axon_ntff_profile_hook() is not None:
                return
        except ImportError:
            pass
        import antenv
        from trn_agent_boot.trn_boot import _ntff_profile_via_ctypes

        hook = _ntff_profile_via_ctypes("/opt/axon/libaxon_pjrt.so")
        mod = types.ModuleType("antenv.axon_hooks")
        mod._hook = hook
        mod.set_axon_ntff_profile_hook = lambda h: setattr(mod, "_hook", h)
        mod.get_axon_ntff_profile_hook = lambda: mod._hook
        sys.modules["antenv.axon_hooks"] = mod
        antenv.axon_hooks = mod

        # artifact upload has no backing store in this container
        from concourse import bass_utils

        bass_utils.upload_artifacts = lambda tmpdir: tmpdir
    except Exception as e:  # pragma: no cover
        print(f"ntff hook registration failed: {e}", file=sys.stderr)


def kernel(xmm, xa, xv, Wq, Wkv, Wproj, bproj, _want_profile=False):
    from concourse.bass_utils import run_bass_kernel_spmd

    if _want_profile:
        _register_ntff_hook()
    nc = _get_program()
    in_maps = _shard_inputs(
        np.asarray(xmm, np.float32), np.asarray(xa, np.float32),
        np.asarray(xv, np.float32), np.asarray(Wq, np.float32),
        np.asarray(Wkv, np.float32), np.asarray(Wproj, np.float32),
    )
    res = run_bass_kernel_spmd(
        nc, in_maps, core_ids=list(range(N_CORES)), trace=_want_profile
    )
    out = np.zeros((B, N_MM, DIM), np.float32)
    for core in range(N_CORES):
        out[core // 4] += res.results[core]["out"]
    out += np.asarray(bproj, np.float32)[None, None, :]
    if _want_profile:
        return out, res
    return out



# revision 22
# speedup vs baseline: 1.1628x; 1.1628x over previous
"""Trainium2 Bass kernel for nn_CrossAttention_DenseAVInteractions.

Math: the reference builds a cartesian KV grid kv[b,i,j] = pv[b,i] + pa[b,j]
over (N_v, N_a) and attends 64 queries against all N_v*N_a = 65536 keys.
Because the logits decompose as s[q,(i,j)] = (q.k_v[i]) + (q.k_a[j]), the
softmax over the product grid factorizes exactly:

    p[q,(i,j)] = softmax_i(q.k_v)[q,i] * softmax_j(q.k_a)[q,j]
    out[q]     = softmax_i(q.k_v) @ v_v + softmax_j(q.k_a) @ v_a

so the whole attention reduces to two 256-key attentions per (b, h).

Sharding (8 cores): core c handles batch b = c // 4 and the head pair
(2j, 2j+1) with j = c % 4.  Each core computes its heads' partial output
projection partial = out_heads @ Wproj[:, head_cols].T; the host sums
the 4 partials per batch and adds bproj.

Device-side design (v2 — rebuilt from the first trace rounds):
 - Everything is bf16 (measured end-to-end rel err ~5e-3 vs the 2e-2 gate),
   halving both HBM traffic (1.37 MB/core) and engine copy time.  fp8 was
   measured too lossy (5.5e-2) and is not used.
 - Scores are computed TRANSPOSED from the start: sT[keys, (h,q)] via
   per-head 64-contract matmuls into PE row-groups.  exp(sT) is then
   directly the PV operand (keys on partitions), eliminating all four
   128x128 p-transposes of the v1 kernel.  The softmax denominator comes
   from a ones-matmul (sum over key partitions), the reciprocal is applied
   per-partition on the tiny o[(h,q), ch] tile after PV.
 - V is projected directly into [keys, ch] layout (8 N=128 matmuls per
   side) instead of [ch, keys] + PE transposes.
 - One [128,64] transpose + 2 [64,64] assembly copies rebuild oA[(h,ch), q]
   for the output projection; the tail is split per head so head 0's
   normalize/transpose overlaps head 1's PV.
 - Inputs stream as 10 chunks over both HWDGE queues (sync + scalar), in
   consumption order; a short cold-PE warmup burst fills the DMA dead time.
"""

import os
import sys

import numpy as np

sys.path.insert(0, "/opt/trn_rl_repo")

DIM = 512
H = 8
HD = DIM // H          # 64
B = 2
N_MM = 64
N_A = 256
N_V = 256
SCALE = HD ** -0.5     # 0.125
N_CORES = 8

# pack column offsets (bf16 columns in the [128, 5376] packed input)
O_WQ = 0          # 4 k-tiles x 128
O_XMM = 512       # 4 x 64
O_WKA = 768       # 4 x 128
O_XA = 1280       # 4 x 256
O_WVA = 2304      # 4 x 128
O_WKV = 2816      # 4 x 128
O_XV = 3328       # 4 x 256
O_WVV = 4352      # 4 x 128
O_WPROJ = 4864    # [128ch, 512]
PACK_COLS = 5376

# chunk boundaries (cols) and which engine issues the load; per-engine
# emission order = HW queue FIFO order.  sync carries q + a-side k-path,
# scalar carries a-side v-weights + v-side; the late-needed wvv/wproj ride
# the gpsimd SWDGE so neither HWDGE sequencer pays their issue cost.
CHUNKS = [
    (O_WQ, 768, "sync"),        # wq + xmm
    (O_WKA, 1280, "sync"),      # wka
    (O_XA, 1792, "sync"),       # xa k0,k1
    (1792, 2304, "sync"),       # xa k2,k3
    (O_WVA, 2816, "scalar"),    # wva
    (O_WKV, 3328, "scalar"),    # wkv
    (O_XV, 3840, "scalar"),     # xv k0,k1
    (3840, 4352, "scalar"),     # xv k2,k3
    (O_WVV, 4864, "gpsimd"),    # wvv
    (O_WPROJ, 5376, "gpsimd"),  # wproj
]

N_WARMUP = 9

_cached = {}


def _build_program():
    import concourse.bacc as bacc
    from concourse import mybir
    from concourse.tile import TileContext

    f32 = mybir.dt.float32
    bf16 = mybir.dt.bfloat16
    nc = bacc.Bacc(name="cross_attn_dense_av2")

    packA = nc.dram_tensor("packA", [128, PACK_COLS], bf16, kind="ExternalInput")
    out_d = nc.dram_tensor("out", [64, 512], bf16, kind="ExternalOutput")
    import ml_dtypes
    ident_d = nc.inline_tensor(
        np.eye(128).astype(ml_dtypes.bfloat16), name="ident128"
    )

    from contextlib import ExitStack

    with TileContext(nc) as tc, ExitStack() as ctx:
        io = ctx.enter_context(tc.tile_pool(name="io", bufs=1))
        work = ctx.enter_context(tc.tile_pool(name="work", bufs=1))
        # PSUM budget is 8 banks, one per pool buffer:
        #   ps_mid (2): warm -> sT_a -> v_a -> sT_v -> v_v rotation
        #   ps_k   (1): kT_a -> kT_v
        #   ps_sm  (3): q -> z_a -> o_a -> z_v -> o_v rotation
        #   ps_f   (1): oT -> f
        ps_mid = ctx.enter_context(tc.tile_pool(name="ps_mid", bufs=2, space="PSUM"))
        ps_k = ctx.enter_context(tc.tile_pool(name="ps_k", bufs=1, space="PSUM"))
        ps_sm = ctx.enter_context(tc.tile_pool(name="ps_sm", bufs=3, space="PSUM"))
        ps_f = ctx.enter_context(tc.tile_pool(name="ps_f", bufs=1, space="PSUM"))

        # ---- loads: chunks in consumption order on both HWDGE queues ----
        ident = io.tile([128, 128], bf16, tag="ident")
        nc.gpsimd.dma_start(out=ident, in_=ident_d[:, :])
        chunk_t = {}
        for lo, hi, eng in CHUNKS:
            t = io.tile([128, hi - lo], bf16, tag=f"c{lo}")
            getattr(nc, eng).dma_start(out=t, in_=packA[:, lo:hi])
            chunk_t[lo] = t

        def col(off, width):
            """AP slice of the packed input at absolute column offset."""
            for lo, hi, _ in CHUNKS:
                if lo <= off and off + width <= hi:
                    return chunk_t[lo][:, off - lo:off - lo + width]
            raise ValueError(f"span {off}:{off + width} crosses chunk boundary")

        # ---- PE warmup: cold matmuls on memset scratch while the first
        #      input chunks are still in flight ----
        warm_sb = io.tile([128, 256], bf16, tag="warm_sb")
        nc.vector.memset(warm_sb, 0.5)
        ones = io.tile([128, 1], bf16, tag="ones")
        nc.vector.memset(ones, 1.0)
        warm_ps = ps_mid.tile([128, 256], f32, tag="mid")
        for w in range(N_WARMUP):
            nc.tensor.matmul(
                warm_ps, warm_sb[:, 0:128], warm_sb,
                start=(w == 0), stop=(w == N_WARMUP - 1),
            )

        # ---- q projection: qT [128(2h,64ch), 64q], scaled on copy-out ----
        q_ps = ps_sm.tile([128, 64], f32, tag="sm")
        for k in range(4):
            nc.tensor.matmul(
                q_ps, col(O_WQ + 128 * k, 128), col(O_XMM + 64 * k, 64),
                start=(k == 0), stop=(k == 3),
            )
        # q2z [128(h,ch), 128(h,q)] block-diagonal: head h's scaled q block at
        # [64h:64h+64, 64h:64h+64].  This lets each scores tile be ONE full
        # 128-contract matmul (the off-head contributions multiply by zero)
        # instead of per-head row-group matmuls.
        q2z = work.tile([128, 128], bf16, tag="q2z")
        nc.vector.memset(q2z, 0.0)
        for h in range(2):
            hs = slice(64 * h, 64 * h + 64)
            nc.vector.tensor_scalar_mul(q2z[hs, 64 * h:64 * h + 64], q_ps[hs, :], SCALE)

        def kproj(o_wk, o_x, side):
            """kT [128(2h,64ch), 256tok] -> SBUF bf16"""
            kp = ps_k.tile([128, 256], f32, tag="k")
            for k in range(4):
                nc.tensor.matmul(
                    kp, col(o_wk + 128 * k, 128), col(o_x + 256 * k, 256),
                    start=(k == 0), stop=(k == 3),
                )
            ks = work.tile([128, 256], bf16, tag=f"k_sb{side}")
            nc.vector.tensor_copy(ks, kp)
            return ks

        def scores(ks, side):
            """sT [128keys(half on free), 2x128(h,q)]; exp -> SBUF bf16"""
            sp = ps_mid.tile([128, 256], f32, tag="mid")
            for half in range(2):
                nc.tensor.matmul(
                    sp[:, 128 * half:128 * half + 128],
                    ks[:, 128 * half:128 * half + 128],
                    q2z,
                    start=True, stop=True,
                )
            ex = work.tile([128, 256], bf16, tag=f"e_sb{side}")
            nc.scalar.activation(ex, sp, mybir.ActivationFunctionType.Exp)
            return ex

        def vproj(o_wv, o_x, side, eng):
            """v [128keys(half on free), 2x128(2h,64ch)] -> SBUF bf16"""
            vp = ps_mid.tile([128, 256], f32, tag="mid")
            for half in range(2):
                for k in range(4):
                    nc.tensor.matmul(
                        vp[:, 128 * half:128 * half + 128],
                        col(o_x + 256 * k + 128 * half, 128),
                        col(o_wv + 128 * k, 128),
                        start=(k == 0), stop=(k == 3),
                    )
            v = work.tile([128, 256], bf16, tag=f"v_sb{side}")
            eng.tensor_copy(v, vp)
            return v

        def zsum(ex, side):
            """z [128(h,q), 1] = sum over keys; reciprocal on DVE -> f32"""
            zp = ps_sm.tile([128, 1], f32, tag="sm")
            for half in range(2):
                nc.tensor.matmul(
                    zp, ex[:, 128 * half:128 * half + 128], ones,
                    start=(half == 0), stop=(half == 1),
                )
            zr = work.tile([128, 1], f32, tag=f"zr{side}")
            nc.vector.reciprocal(zr, zp)
            return zr

        def pv(ex, v, side):
            """o [128(h,q), 64ch] PSUM, per-head accumulation groups"""
            op = ps_sm.tile([128, 64], f32, tag="sm")
            for h in range(2):
                for half in range(2):
                    hc = 128 * half + 64 * h
                    nc.tensor.matmul(
                        op[64 * h:64 * h + 64, :],
                        ex[:, hc:hc + 64],
                        v[:, hc:hc + 64],
                        start=(half == 0), stop=(half == 1),
                        tile_position=(0, 64 * h),
                    )
            return op

        # ---- pipelined compute, emitted in expected execution order ----
        # staged build for bisection: 10=q, 11=+kproj_a, 12=+scores_a(noexp),
        # 13=+exp_a, 1=both sides through exp, 2=+z/PV, 3=full
        KLEVEL = int(os.environ.get("KLEVEL", "3"))

        if KLEVEL == 10:
            qs = work.tile([128, 64], bf16, tag="qs")
            nc.scalar.copy(qs, q_ps)
            nc.sync.dma_start(out=out_d[:, 0:64], in_=qs[0:64, :])
        if KLEVEL >= 11:
            ks_a = kproj(O_WKA, O_XA, 0)
        if KLEVEL == 11:
            nc.sync.dma_start(out=out_d[:, 0:256], in_=ks_a[0:64, :])
        if KLEVEL == 12:
            sp = ps_mid.tile([128, 256], f32, tag="mid")
            for half in range(2):
                nc.tensor.matmul(
                    sp[:, 128 * half:128 * half + 128],
                    ks_a[:, 128 * half:128 * half + 128],
                    q2z,
                    start=True, stop=True,
                )
            sd = work.tile([128, 256], bf16, tag="sd")
            nc.vector.tensor_copy(sd, sp)
            nc.sync.dma_start(out=out_d[:, 0:256], in_=sd[0:64, :])
        if KLEVEL >= 13:
            exp_a = scores(ks_a, 0)
        if KLEVEL == 13:
            nc.sync.dma_start(out=out_d[:, 0:256], in_=exp_a[0:64, :])
        if KLEVEL >= 1 and KLEVEL < 10:
            ks_a = kproj(O_WKA, O_XA, 0)
            exp_a = scores(ks_a, 0)
            v_a = vproj(O_WVA, O_XA, 0, nc.vector)
            ks_v = kproj(O_WKV, O_XV, 1)
            exp_v = scores(ks_v, 1)
        if KLEVEL == 1:
            nc.sync.dma_start(out=out_d[:, 0:256], in_=exp_a[0:64, 0:256])
            nc.scalar.dma_start(out=out_d[:, 256:512], in_=exp_v[0:64, 0:256])
        if KLEVEL >= 2 and KLEVEL < 10:
            zr_a = zsum(exp_a, 0)
            o_a = pv(exp_a, v_a, 0)
            zr_v = zsum(exp_v, 1)
            v_v = vproj(O_WVV, O_XV, 1, nc.vector)
            o_v = pv(exp_v, v_v, 1)
        if KLEVEL == 2:
            t_a = work.tile([128, 64], bf16, tag="t_a")
            nc.vector.tensor_copy(t_a, o_a)
            t_v = work.tile([128, 64], bf16, tag="t_v")
            nc.scalar.copy(t_v, o_v)
            nc.sync.dma_start(out=out_d[:, 0:64], in_=t_a[0:64, :])
            nc.scalar.dma_start(out=out_d[:, 64:128], in_=t_v[0:64, :])

        # ---- per-head: normalize both sides, add, transpose, assemble.
        # The transpose of head h's [64(h,q), 64ch] block is placed at PSUM
        # partitions 64h so the oA copies stay partition-aligned. ----
        if KLEVEL == 3:
            oA = work.tile([128, 64], bf16, tag="oA")
            on_a = work.tile([128, 64], f32, tag="on_a")
            on_v = work.tile([128, 64], f32, tag="on_v")
            oc = work.tile([128, 64], bf16, tag="oc")
            oT_ps = ps_f.tile([128, 64], bf16, tag="f")
            for h in range(2):
                hs = slice(64 * h, 64 * h + 64)
                nc.vector.tensor_scalar_mul(on_a[hs, :], o_a[hs, :], zr_a[hs, :])
                nc.scalar.mul(on_v[hs, :], o_v[hs, :], zr_v[hs, :])
                nc.vector.tensor_add(oc[hs, :], on_a[hs, :], on_v[hs, :])
                nc.tensor.transpose(
                    oT_ps[hs, :], oc[hs, :], ident[hs, hs],
                    tile_position=(64 * h, 64 * h),
                )
                nc.scalar.copy(oA[hs, :], oT_ps[hs, :])

            # ---- output projection partial: [64q, 512], split for overlap ----
            f_ps = ps_f.tile([64, 512], f32, tag="f")
            f_sb = work.tile([64, 512], bf16, tag="f_sb")
            for half in range(2):
                cs = slice(256 * half, 256 * half + 256)
                nc.tensor.matmul(
                    f_ps[:, cs], oA, col(O_WPROJ + 256 * half, 256),
                    start=True, stop=True,
                )
                (nc.vector.tensor_copy if half == 0 else nc.scalar.copy)(
                    f_sb[:, cs], f_ps[:, cs]
                )
                getattr(nc, "sync" if half == 0 else "scalar").dma_start(
                    out=out_d[:, cs], in_=f_sb[:, cs]
                )

    nc.finalize()
    return nc


def _km(a):
    """[512, C] K-major -> [128, 4*C] (4 k-tiles side by side)."""
    c = a.shape[1]
    return a.reshape(4, 128, c).transpose(1, 0, 2).reshape(128, 4 * c)


def _shard_inputs(xmm, xa, xv, Wq, Wkv, Wproj):
    """Build the 8 per-core input maps (one packed [128, 5376] bf16 each)."""
    import ml_dtypes

    in_maps = []
    for core in range(N_CORES):
        b, j = divmod(core, 4)
        r = slice(128 * j, 128 * j + 128)               # head-pair rows in [0,512)
        rv = slice(512 + 128 * j, 512 + 128 * j + 128)  # v rows in Wkv
        pack = np.concatenate(
            [
                _km(Wq[r, :].T),            # O_WQ
                _km(xmm[b].T),              # O_XMM
                _km(Wkv[r, 512:].T),        # O_WKA
                _km(xa[b].T),               # O_XA
                _km(Wkv[rv, 512:].T),       # O_WVA
                _km(Wkv[r, :512].T),        # O_WKV
                _km(xv[b].T),               # O_XV
                _km(Wkv[rv, :512].T),       # O_WVV
                Wproj[:, 128 * j:128 * j + 128].T,  # O_WPROJ
            ],
            axis=1,
        )
        assert pack.shape == (128, PACK_COLS)
        in_maps.append(
            {"packA": np.ascontiguousarray(pack).astype(ml_dtypes.bfloat16)}
        )
    return in_maps


def _get_program():
    if "nc" not in _cached:
        _cached["nc"] = _build_program()
    return _cached["nc"]


def _register_ntff_hook():
    """Best-effort: register the axon NTFF profile hook that the container's
    antenv stub doesn't provide, so run_bass_kernel_spmd(trace=True) can
    measure HW exec time. No-op on failure."""
    try:
        import types

        try:
            from antenv.axon_hooks import get_axon_ntff_profile_hook
            if get_axon_ntff_profile_hook() is not None:
                return
        except ImportError:
            pass
        import antenv
        from trn_agent_boot.trn_boot import _ntff_profile_via_ctypes

        hook = _ntff_profile_via_ctypes("/opt/axon/libaxon_pjrt.so")
        mod = types.ModuleType("antenv.axon_hooks")
        mod._hook = hook
        mod.set_axon_ntff_profile_hook = lambda h: setattr(mod, "_hook", h)
        mod.get_axon_ntff_profile_hook = lambda: mod._hook
        sys.modules["antenv.axon_hooks"] = mod
        antenv.axon_hooks = mod

        # artifact upload has no backing store in this container
        from concourse import bass_utils

        bass_utils.upload_artifacts = lambda tmpdir: tmpdir
    except Exception as e:  # pragma: no cover
        print(f"ntff hook registration failed: {e}", file=sys.stderr)


def kernel(xmm, xa, xv, Wq, Wkv, Wproj, bproj, _want_profile=False):
    from concourse.bass_utils import run_bass_kernel_spmd

    if _want_profile:
        _register_ntff_hook()
    nc = _get_program()
    in_maps = _shard_inputs(
        np.asarray(xmm, np.float32), np.asarray(xa, np.float32),
        np.asarray(xv, np.float32), np.asarray(Wq, np.float32),
        np.asarray(Wkv, np.float32), np.asarray(Wproj, np.float32),
    )
    res = run_bass_kernel_spmd(
        nc, in_maps, core_ids=list(range(N_CORES)), trace=_want_profile
    )
    out = np.zeros((B, N_MM, DIM), np.float32)
    for core in range(N_CORES):
        out[core // 4] += np.asarray(res.results[core]["out"], np.float32)
    out += np.asarray(bproj, np.float32)[None, None, :]
    if _want_profile:
        return out, res
    return out


# revision 25
# speedup vs baseline: 1.2208x; 1.0498x over previous
"""Trainium2 Bass kernel for nn_CrossAttention_DenseAVInteractions.

Math: the reference builds a cartesian KV grid kv[b,i,j] = pv[b,i] + pa[b,j]
over (N_v, N_a) and attends 64 queries against all N_v*N_a = 65536 keys.
Because the logits decompose as s[q,(i,j)] = (q.k_v[i]) + (q.k_a[j]), the
softmax over the product grid factorizes exactly:

    p[q,(i,j)] = softmax_i(q.k_v)[q,i] * softmax_j(q.k_a)[q,j]
    out[q]     = softmax_i(q.k_v) @ v_v + softmax_j(q.k_a) @ v_a

so the whole attention reduces to two 256-key attentions per (b, h).

Sharding (8 cores): core c handles batch b = c // 4 and the head pair
(2j, 2j+1) with j = c % 4.  Each core computes its heads' partial output
projection partial = out_heads @ Wproj[:, head_cols].T; the host sums
the 4 partials per batch and adds bproj.

Device-side design (v2 — rebuilt from the first trace rounds):
 - Everything is bf16 (measured end-to-end rel err ~5e-3 vs the 2e-2 gate),
   halving both HBM traffic (1.37 MB/core) and engine copy time.  fp8 was
   measured too lossy (5.5e-2) and is not used.
 - Scores are computed TRANSPOSED from the start: sT[keys, (h,q)] via
   per-head 64-contract matmuls into PE row-groups.  exp(sT) is then
   directly the PV operand (keys on partitions), eliminating all four
   128x128 p-transposes of the v1 kernel.  The softmax denominator comes
   from a ones-matmul (sum over key partitions), the reciprocal is applied
   per-partition on the tiny o[(h,q), ch] tile after PV.
 - V is projected directly into [keys, ch] layout (8 N=128 matmuls per
   side) instead of [ch, keys] + PE transposes.
 - One [128,64] transpose + 2 [64,64] assembly copies rebuild oA[(h,ch), q]
   for the output projection; the tail is split per head so head 0's
   normalize/transpose overlaps head 1's PV.
 - Inputs stream as 10 chunks over both HWDGE queues (sync + scalar), in
   consumption order; a short cold-PE warmup burst fills the DMA dead time.
"""

import os
import sys

import numpy as np

sys.path.insert(0, "/opt/trn_rl_repo")

DIM = 512
H = 8
HD = DIM // H          # 64
B = 2
N_MM = 64
N_A = 256
N_V = 256
SCALE = HD ** -0.5     # 0.125
N_CORES = 8

# pack column offsets (bf16 columns in the [128, 5376] packed input)
O_WQ = 0          # 4 k-tiles x 128
O_XMM = 512       # 4 x 64
O_WKA = 768       # 4 x 128
O_XA = 1280       # 4 x 256
O_WVA = 2304      # 4 x 128
O_WKV = 2816      # 4 x 128
O_XV = 3328       # 4 x 256
O_WVV = 4352      # 4 x 128
O_WPROJ = 4864    # [128ch, 512]
PACK_COLS = 5376

# chunk boundaries (cols) and which engine issues the load; per-engine
# emission order = HW queue FIFO order.  sync carries q + a-side k-path,
# scalar carries a-side v-weights + v-side; the late-needed wvv/wproj ride
# the gpsimd SWDGE so neither HWDGE sequencer pays their issue cost.
CHUNKS = [
    (O_WKA, 1280, "sync"),      # wka        (kproj_a lhs, first on q0)
    (O_XA, 1792, "scalar"),     # xa k0,k1   (kproj_a rhs, first on q1)
    (1792, 2304, "sync"),       # xa k2,k3
    (O_WQ, 768, "sync"),        # wq + xmm
    (O_WVA, 2816, "scalar"),    # wva
    (O_WKV, 3328, "scalar"),    # wkv
    (O_XV, 3840, "scalar"),     # xv k0,k1
    (3840, 4352, "sync"),       # xv k2,k3
    (O_WVV, 4864, "gpsimd"),    # wvv
    (O_WPROJ, 5376, "gpsimd"),  # wproj
]

N_WARMUP = 5

_cached = {}


def _build_program():
    import concourse.bacc as bacc
    from concourse import mybir
    from concourse.tile import TileContext

    f32 = mybir.dt.float32
    bf16 = mybir.dt.bfloat16
    nc = bacc.Bacc(name="cross_attn_dense_av2")

    packA = nc.dram_tensor("packA", [128, PACK_COLS], bf16, kind="ExternalInput")
    out_d = nc.dram_tensor("out", [64, 512], bf16, kind="ExternalOutput")
    import ml_dtypes
    ident_d = nc.inline_tensor(
        np.eye(128).astype(ml_dtypes.bfloat16), name="ident128"
    )

    from contextlib import ExitStack

    with TileContext(nc) as tc, ExitStack() as ctx:
        io = ctx.enter_context(tc.tile_pool(name="io", bufs=1))
        work = ctx.enter_context(tc.tile_pool(name="work", bufs=1))
        # PSUM budget is 8 banks, one per pool buffer:
        #   ps_mid (2): warm -> sT_a -> v_a -> sT_v -> v_v rotation
        #   ps_k   (1): kT_a -> kT_v
        #   ps_sm  (3): q -> z_a -> o_a -> z_v -> o_v rotation
        #   ps_f   (1): oT -> f
        ps_mid = ctx.enter_context(tc.tile_pool(name="ps_mid", bufs=2, space="PSUM"))
        ps_k = ctx.enter_context(tc.tile_pool(name="ps_k", bufs=1, space="PSUM"))
        ps_sm = ctx.enter_context(tc.tile_pool(name="ps_sm", bufs=3, space="PSUM"))
        ps_f = ctx.enter_context(tc.tile_pool(name="ps_f", bufs=2, space="PSUM"))

        # ---- loads: chunks in consumption order on both HWDGE queues ----
        ident = io.tile([128, 128], bf16, tag="ident")
        nc.gpsimd.dma_start(out=ident, in_=ident_d[:, :])
        chunk_t = {}
        for lo, hi, eng in CHUNKS:
            t = io.tile([128, hi - lo], bf16, tag=f"c{lo}")
            getattr(nc, eng).dma_start(out=t, in_=packA[:, lo:hi])
            chunk_t[lo] = t

        def col(off, width):
            """AP slice of the packed input at absolute column offset."""
            for lo, hi, _ in CHUNKS:
                if lo <= off and off + width <= hi:
                    return chunk_t[lo][:, off - lo:off - lo + width]
            raise ValueError(f"span {off}:{off + width} crosses chunk boundary")

        # ---- PE warmup: cold matmuls on memset scratch while the first
        #      input chunks are still in flight ----
        warm_sb = io.tile([128, 256], bf16, tag="warm_sb")
        nc.vector.memset(warm_sb, 0.5)
        ones = io.tile([128, 1], bf16, tag="ones")
        nc.vector.memset(ones, 1.0)
        warm_ps = ps_mid.tile([128, 256], f32, tag="mid")
        for w in range(N_WARMUP):
            nc.tensor.matmul(
                warm_ps, warm_sb[:, 0:128], warm_sb,
                start=(w == 0), stop=(w == N_WARMUP - 1),
            )

        def kproj(o_wk, o_x, side):
            """kT [128(2h,64ch), 256tok] -> SBUF bf16"""
            kp = ps_k.tile([128, 256], f32, tag="k")
            for k in range(4):
                nc.tensor.matmul(
                    kp, col(o_wk + 128 * k, 128), col(o_x + 256 * k, 256),
                    start=(k == 0), stop=(k == 3),
                )
            ks = work.tile([128, 256], bf16, tag=f"k_sb{side}")
            nc.vector.tensor_copy(ks, kp)
            return ks

        def scores(ks, side):
            """sT [128keys(half on free), 2x128(h,q)]; exp -> SBUF bf16"""
            sp = ps_mid.tile([128, 256], f32, tag="mid")
            for half in range(2):
                nc.tensor.matmul(
                    sp[:, 128 * half:128 * half + 128],
                    ks[:, 128 * half:128 * half + 128],
                    q2z,
                    start=True, stop=True,
                )
            ex = work.tile([128, 256], bf16, tag=f"e_sb{side}")
            nc.scalar.activation(ex, sp, mybir.ActivationFunctionType.Exp)
            return ex

        def vproj(o_wv, o_x, side, eng):
            """v [128keys(half on free), 2x128(2h,64ch)] -> SBUF bf16"""
            vp = ps_mid.tile([128, 256], f32, tag="mid")
            for half in range(2):
                for k in range(4):
                    nc.tensor.matmul(
                        vp[:, 128 * half:128 * half + 128],
                        col(o_x + 256 * k + 128 * half, 128),
                        col(o_wv + 128 * k, 128),
                        start=(k == 0), stop=(k == 3),
                    )
            v = work.tile([128, 256], bf16, tag=f"v_sb{side}")
            eng.tensor_copy(v, vp)
            return v

        def zsum(ex, side):
            """z [128(h,q), 1] = sum over keys; reciprocal on DVE -> f32"""
            zp = ps_sm.tile([128, 1], f32, tag="sm")
            for half in range(2):
                nc.tensor.matmul(
                    zp, ex[:, 128 * half:128 * half + 128], ones,
                    start=(half == 0), stop=(half == 1),
                )
            zr = work.tile([128, 1], f32, tag=f"zr{side}")
            nc.vector.reciprocal(zr, zp)
            return zr

        def pv(ex, v, side):
            """o [128(h,q), 64ch] PSUM, per-head accumulation groups"""
            op = ps_sm.tile([128, 64], f32, tag="sm")
            for h in range(2):
                for half in range(2):
                    hc = 128 * half + 64 * h
                    nc.tensor.matmul(
                        op[64 * h:64 * h + 64, :],
                        ex[:, hc:hc + 64],
                        v[:, hc:hc + 64],
                        start=(half == 0), stop=(half == 1),
                        tile_position=(0, 64 * h),
                    )
            return op

        # ---- pipelined compute, emitted in expected execution order ----
        # ---- a-side kproj first: its chunks lead both queues ----
        ks_a = kproj(O_WKA, O_XA, 0)

        # ---- q projection: qT [128(2h,64ch), 64q] ----
        q_ps = ps_sm.tile([128, 64], f32, tag="sm")
        for k in range(4):
            nc.tensor.matmul(
                q_ps, col(O_WQ + 128 * k, 128), col(O_XMM + 64 * k, 64),
                start=(k == 0), stop=(k == 3),
            )
        # q2z [128(h,ch), 128(h,q)] block-diagonal: head h's scaled q block at
        # [64h:64h+64, 64h:64h+64].  This lets each scores tile be ONE full
        # 128-contract matmul (the off-head contributions multiply by zero)
        # instead of per-head row-group matmuls (which hang the PE).
        q2z = work.tile([128, 128], bf16, tag="q2z")
        nc.vector.memset(q2z, 0.0)
        for h in range(2):
            hs = slice(64 * h, 64 * h + 64)
            nc.vector.tensor_scalar_mul(q2z[hs, 64 * h:64 * h + 64], q_ps[hs, :], SCALE)

        exp_a = scores(ks_a, 0)
        v_a = vproj(O_WVA, O_XA, 0, nc.vector)
        ks_v = kproj(O_WKV, O_XV, 1)
        exp_v = scores(ks_v, 1)
        v_v = vproj(O_WVV, O_XV, 1, nc.vector)
        zr_a = zsum(exp_a, 0)
        o_a = pv(exp_a, v_a, 0)
        zr_v = zsum(exp_v, 1)
        o_v = pv(exp_v, v_v, 1)

        # ---- per-head: normalize both sides, add, transpose, assemble.
        # The transpose of head h's [64(h,q), 64ch] block is placed at PSUM
        # partitions 64h so the oA copies stay partition-aligned.  The a-side
        # muls are hoisted so they run as soon as o_a/zr_a exist. ----
        oA = work.tile([128, 64], bf16, tag="oA")
        on_a = work.tile([128, 64], f32, tag="on_a")
        on_v = work.tile([128, 64], f32, tag="on_v")
        oc = work.tile([128, 64], bf16, tag="oc")
        oT_ps = ps_f.tile([128, 64], bf16, tag="f")
        nc.vector.tensor_scalar_mul(on_a, o_a, zr_a)
        for h in range(2):
            hs = slice(64 * h, 64 * h + 64)
            nc.scalar.mul(on_v[hs, :], o_v[hs, :], zr_v[hs, :])
            nc.vector.tensor_add(oc[hs, :], on_a[hs, :], on_v[hs, :])
            nc.tensor.transpose(
                oT_ps[hs, :], oc[hs, :], ident[hs, hs],
                tile_position=(64 * h, 64 * h),
            )
            nc.scalar.copy(oA[hs, :], oT_ps[hs, :])

        # ---- output projection partial: [64q, 512], halves in separate
        # PSUM banks so half 1's matmul doesn't wait on half 0's copy ----
        f_sb = work.tile([64, 512], bf16, tag="f_sb")
        for half in range(2):
            cs = slice(256 * half, 256 * half + 256)
            f_ps = ps_f.tile([64, 256], f32, tag="f")
            nc.tensor.matmul(
                f_ps, oA, col(O_WPROJ + 256 * half, 256),
                start=True, stop=True,
            )
            (nc.vector.tensor_copy if half == 0 else nc.scalar.copy)(
                f_sb[:, cs], f_ps
            )
            getattr(nc, "sync" if half == 0 else "scalar").dma_start(
                out=out_d[:, cs], in_=f_sb[:, cs]
            )

    nc.finalize()
    return nc


def _km(a):
    """[512, C] K-major -> [128, 4*C] (4 k-tiles side by side)."""
    c = a.shape[1]
    return a.reshape(4, 128, c).transpose(1, 0, 2).reshape(128, 4 * c)


def _shard_inputs(xmm, xa, xv, Wq, Wkv, Wproj):
    """Build the 8 per-core input maps (one packed [128, 5376] bf16 each)."""
    import ml_dtypes

    in_maps = []
    for core in range(N_CORES):
        b, j = divmod(core, 4)
        r = slice(128 * j, 128 * j + 128)               # head-pair rows in [0,512)
        rv = slice(512 + 128 * j, 512 + 128 * j + 128)  # v rows in Wkv
        pack = np.concatenate(
            [
                _km(Wq[r, :].T),            # O_WQ
                _km(xmm[b].T),              # O_XMM
                _km(Wkv[r, 512:].T),        # O_WKA
                _km(xa[b].T),               # O_XA
                _km(Wkv[rv, 512:].T),       # O_WVA
                _km(Wkv[r, :512].T),        # O_WKV
                _km(xv[b].T),               # O_XV
                _km(Wkv[rv, :512].T),       # O_WVV
                Wproj[:, 128 * j:128 * j + 128].T,  # O_WPROJ
            ],
            axis=1,
        )
        assert pack.shape == (128, PACK_COLS)
        in_maps.append(
            {"packA": np.ascontiguousarray(pack).astype(ml_dtypes.bfloat16)}
        )
    return in_maps


def _get_program():
    if "nc" not in _cached:
        _cached["nc"] = _build_program()
    return _cached["nc"]


def _register_ntff_hook():
    """Best-effort: register the axon NTFF profile hook that the container's
    antenv stub doesn't provide, so run_bass_kernel_spmd(trace=True) can
    measure HW exec time. No-op on failure."""
    try:
        import types

        try:
            from antenv.axon_hooks import get_axon_ntff_profile_hook
            if get_axon_ntff_profile_hook() is not None:
                return
        except ImportError:
            pass
        import antenv
        from trn_agent_boot.trn_boot import _ntff_profile_via_ctypes

        hook = _ntff_profile_via_ctypes("/opt/axon/libaxon_pjrt.so")
        mod = types.ModuleType("antenv.axon_hooks")
        mod._hook = hook
        mod.set_axon_ntff_profile_hook = lambda h: setattr(mod, "_hook", h)
        mod.get_axon_ntff_profile_hook = lambda: mod._hook
        sys.modules["antenv.axon_hooks"] = mod
        antenv.axon_hooks = mod

        # artifact upload has no backing store in this container
        from concourse import bass_utils

        bass_utils.upload_artifacts = lambda tmpdir: tmpdir
    except Exception as e:  # pragma: no cover
        print(f"ntff hook registration failed: {e}", file=sys.stderr)


def kernel(xmm, xa, xv, Wq, Wkv, Wproj, bproj, _want_profile=False):
    from concourse.bass_utils import run_bass_kernel_spmd

    if _want_profile:
        _register_ntff_hook()
    nc = _get_program()
    in_maps = _shard_inputs(
        np.asarray(xmm, np.float32), np.asarray(xa, np.float32),
        np.asarray(xv, np.float32), np.asarray(Wq, np.float32),
        np.asarray(Wkv, np.float32), np.asarray(Wproj, np.float32),
    )
    res = run_bass_kernel_spmd(
        nc, in_maps, core_ids=list(range(N_CORES)), trace=_want_profile
    )
    out = np.zeros((B, N_MM, DIM), np.float32)
    for core in range(N_CORES):
        out[core // 4] += np.asarray(res.results[core]["out"], np.float32)
    out += np.asarray(bproj, np.float32)[None, None, :]
    if _want_profile:
        return out, res
    return out
